# revision 6
# baseline (speedup 1.0000x reference)
"""Trainium2 Bass kernel for nn_GB_Flash_Classifier (GAU/FLASH-style classifier).

Sharding: 8 cores = 4 batches x 2 sequence halves (4096 tokens/core).
The causal linear-attention cross-half prefix is exchanged with a pairwise
AllReduce of the local sum of K^T V ([64,512] f32) each layer.

The residual stream h stays SBUF-resident across all 8 layers in fp32
token-major layout [128, 32, 257] (col 256 = running row-sum, making the LN
mean free).  Weights stream per layer from HBM in bf16.  z is transposed to
feature-major with DMA-engine transposes (bf16).
"""
import sys
import math

sys.path.insert(0, "/opt/trn_rl_repo")

import numpy as np
import ml_dtypes

BF16NP = ml_dtypes.bfloat16

# model constants (must match reference.py)
NL_FULL, GROUP, QKD, NB, MAXD = 8, 256, 64, 32, 128
B, N, IN, D, OUT = 4, 8192, 5, 256, 2
HID = 512
MU = math.sqrt(0.5)
STD = math.sqrt(0.25 * math.pi)
NCORES = 8
S0 = 1.0 / (GROUP * STD * math.sqrt(2.0))
TK_FULL = 32  # token tiles of 128 per core (half sequence)


def _rp_bucket_np(g):
    pos = np.arange(g)
    rel = pos[None, :] - pos[:, None]  # rel[q, k] = k - q
    n = np.maximum(-rel, 0)
    me = NB // 2
    nf = np.maximum(n, 1).astype(np.float32)
    val = me + (np.log(nf / me) / math.log(MAXD / me) * (NB - me)).astype(np.int32)
    return np.where(n < me, n, np.minimum(val, NB - 1))


_BUILD_CACHE = {}


def build(nl, tk):
    key = (nl, tk)
    if key in _BUILD_CACHE:
        return _BUILD_CACHE[key]
    from contextlib import ExitStack

    import concourse.bacc as bacc
    import concourse.mybir as mybir
    import concourse.tile as tile

    f32, bf16 = mybir.dt.float32, mybir.dt.bfloat16
    ALU = mybir.AluOpType
    AF = mybir.ActivationFunctionType
    AX = mybir.AxisListType
    nt = tk * 128
    gk = tk // 2
    RG = [[2 * i, 2 * i + 1] for i in range(NCORES // 2)]

    nc = bacc.Bacc("TRN2", target_bir_lowering=False, debug=False, num_devices=NCORES)

    xT_d = nc.dram_tensor("xT", [IN, nt], f32, kind="ExternalInput")
    pos_d = nc.dram_tensor("pos", [nt, 257], f32, kind="ExternalInput")
    wemb_d = nc.dram_tensor("wemb", [IN, 257], f32, kind="ExternalInput")
    ident_d = nc.dram_tensor("ident", [128, 128], f32, kind="ExternalInput")
    onesr_d = nc.dram_tensor("onesr", [1, 512], f32, kind="ExternalInput")
    onesc_d = nc.dram_tensor("onesc", [128, 1], f32, kind="ExternalInput")
    m01_d = nc.dram_tensor("m01", [128, 384], bf16, kind="ExternalInput")
    par_d = nc.dram_tensor("par", [64, 1], f32, kind="ExternalInput")
    pin_d = nc.dram_tensor("pin", [64, 1], f32, kind="ExternalInput")
    whp_d = nc.dram_tensor("whp", [nl, 128, 2, 1024], bf16, kind="ExternalInput")
    wqkp_d = nc.dram_tensor("wqkp", [nl, 128, 2, 64], bf16, kind="ExternalInput")
    wop_d = nc.dram_tensor("wop", [nl, 128, 4, 257], bf16, kind="ExternalInput")
    b2t_d = nc.dram_tensor("b2t", [nl, 128, 2, 256], f32, kind="ExternalInput")
    rows_d = nc.dram_tensor("rows", [nl, 1, 1345], f32, kind="ExternalInput")
    gbh_d = nc.dram_tensor("gbh", [nl, 64, 8], f32, kind="ExternalInput")
    g3b_d = nc.dram_tensor("g3b", [nl, 128, 512], bf16, kind="ExternalInput")
    out_d = nc.dram_tensor("out", [1, 257], f32, kind="ExternalOutput")

    with tile.TileContext(nc) as tc:
        with ExitStack() as ctx:
            const = ctx.enter_context(tc.tile_pool(name="const", bufs=1))
            hp = ctx.enter_context(tc.tile_pool(name="hp", bufs=1))
            wp = ctx.enter_context(tc.tile_pool(name="wp", bufs=2))
            ap = ctx.enter_context(tc.tile_pool(name="ap", bufs=1))
            sp = ctx.enter_context(tc.tile_pool(name="sp", bufs=3))
            st = ctx.enter_context(tc.tile_pool(name="st", bufs=2))
            psA = ctx.enter_context(tc.tile_pool(name="psA", bufs=3, space="PSUM"))
            psB = ctx.enter_context(tc.tile_pool(name="psB", bufs=2, space="PSUM"))
            psL = ctx.enter_context(tc.tile_pool(name="psL", bufs=1, space="PSUM"))
            dr = ctx.enter_context(tc.tile_pool(name="dr", bufs=2, space="DRAM"))

            ident = const.tile([128, 128], f32, tag="ident")
            nc.sync.dma_start(ident[:], ident_d[:])
            onesr = const.tile([1, 512], f32, tag="onesr")
            nc.sync.dma_start(onesr[:], onesr_d[:])
            onesc = const.tile([128, 1], f32, tag="onesc")
            nc.sync.dma_start(onesc[:], onesc_d[:])
            m01 = const.tile([128, 384], bf16, tag="m01")
            nc.sync.dma_start(m01[:], m01_d[:])
            par = const.tile([64, 1], f32, tag="par")
            nc.sync.dma_start(par[:], par_d[:])
            pin = const.tile([64, 1], f32, tag="pin")
            nc.sync.dma_start(pin[:], pin_d[:])
            wemb = const.tile([IN, 257], f32, tag="wemb")
            nc.sync.dma_start(wemb[:], wemb_d[:])

            h = hp.tile([128, tk, 257], f32, tag="h")

            # ---- embedding: h0 = x @ W_emb(+rowsum col) + (pos + b_emb)(+sum col)
            for t in range(tk):
                xtt = sp.tile([IN, 128], f32, tag="xtt")
                nc.sync.dma_start(xtt[:], xT_d[:, t * 128 : (t + 1) * 128])
                pe = sp.tile([128, 257], f32, tag="pos")
                nc.sync.dma_start(pe[:], pos_d[t * 128 : (t + 1) * 128, :])
                ps = psA.tile([128, 512], f32, tag="bank")
                nc.tensor.matmul(
                    ps[:, 0:257],
                    xtt[:],
                    wemb[:],
                    start=True,
                    stop=True,
                )
                nc.vector.tensor_tensor(h[:, t, :], ps[:, 0:257], pe[:], ALU.add)

            for l in range(nl):
                whp = wp.tile([128, 2, 1024], bf16, tag="whp")
                nc.sync.dma_start(whp[:], whp_d[l])
                wqkp = wp.tile([128, 2, 64], bf16, tag="wqkp")
                nc.sync.dma_start(wqkp[:], wqkp_d[l])
                wop = wp.tile([128, 4, 257], bf16, tag="wop")
                nc.sync.dma_start(wop[:], wop_d[l])
                b2t = wp.tile([128, 2, 256], f32, tag="b2t")
                nc.sync.dma_start(b2t[:], b2t_d[l])
                rows = wp.tile([1, 1345], f32, tag="rows")
                nc.sync.dma_start(rows[:], rows_d[l])
                gbh = wp.tile([64, 8], f32, tag="gbh")
                nc.sync.dma_start(gbh[:], gbh_d[l])
                g3b = wp.tile([128, 512], bf16, tag="g3b")
                nc.sync.dma_start(g3b[:], g3b_d[l])
                bhv = rows[:, 0:512]
                bhgr = rows[:, 512:1024]
                bqkr = rows[:, 1024:1088]
                bor = rows[:, 1088:1345]

                # ---- LN stats (mean is free: h[:, :, 256] is the row-sum)
                scr = ap.tile([128, tk, 256], f32, tag="big")
                nc.gpsimd.tensor_tensor(
                    scr[:], h[:, :, 0:256], h[:, :, 0:256], ALU.mult
                )
                sumsq = st.tile([128, tk], f32, tag="sumsq")
                nc.vector.tensor_reduce(sumsq[:], scr[:], axis=AX.X, op=ALU.add)
                negmu = st.tile([128, tk], f32, tag="negmu")
                nc.vector.tensor_scalar_mul(negmu[:], h[:, :, 256], -1.0 / D)
                var = st.tile([128, tk], f32, tag="var")
                nc.vector.tensor_scalar(
                    var[:], sumsq[:], 1.0 / D, 1e-5, ALU.mult, ALU.add
                )
                musq = st.tile([128, tk], f32, tag="musq")
                nc.vector.tensor_tensor(musq[:], negmu[:], negmu[:], ALU.mult)
                nc.vector.tensor_tensor(var[:], var[:], musq[:], ALU.subtract)
                sd = st.tile([128, tk], f32, tag="sd")
                nc.scalar.activation(sd[:], var[:], AF.Sqrt)
                rstd = st.tile([128, tk], f32, tag="rstd")
                nc.vector.reciprocal(rstd[:], sd[:])
                nmr = st.tile([128, tk], f32, tag="nmr")
                nc.vector.tensor_tensor(nmr[:], negmu[:], rstd[:], ALU.mult)

                # ---- z (bf16) and zT via DMA transpose
                zT = ap.tile([128, 2, nt], bf16, tag="zT")
                for t in range(tk):
                    zt = sp.tile([128, 256], bf16, tag="z")
                    nc.vector.tensor_scalar(
                        zt[:],
                        h[:, t, 0:256],
                        rstd[:, t : t + 1],
                        nmr[:, t : t + 1],
                        ALU.mult,
                        ALU.add,
                    )
                    for kc in range(2):
                        nc.sync.dma_start_transpose(
                            zT[:, kc, t * 128 : (t + 1) * 128],
                            zt[:, kc * 128 : (kc + 1) * 128],
                        )

                # ---- qk base (feature-major silu) + heads qq/lq/kk (all base-0)
                qkb = ap.tile([64, nt], bf16, tag="qkb")
                qqv = ap.tile([64, nt], bf16, tag="qqv")
                lqv = ap.tile([64, nt], bf16, tag="lqv")
                kkv = ap.tile([64, nt], bf16, tag="kkv")
                for n2 in range(nt // 1024):
                    ps = psB.tile([64, 2, 512], f32, tag="pair")
                    for i in range(2):
                        off = n2 * 1024 + i * 512
                        for kc in range(2):
                            nc.tensor.matmul(
                                ps[:, i, :],
                                wqkp[:, kc, :],
                                zT[:, kc, off : off + 512],
                                start=(kc == 0),
                                stop=(kc == 1),
                                skip_group_check=True,
                            )
                    nc.scalar.activation(
                        qkb[:, n2 * 1024 : (n2 + 1) * 1024].rearrange(
                            "p (a b) -> p a b", a=2
                        ),
                        ps[:],
                        AF.Silu,
                        bias=gbh[:, 6:7],
                    )
                nc.vector.tensor_scalar(
                    qqv[:], qkb[:], gbh[:, 0:1], gbh[:, 1:2], ALU.mult, ALU.add
                )
                nc.vector.tensor_scalar(
                    lqv[:], qkb[:], gbh[:, 2:3], gbh[:, 3:4], ALU.mult, ALU.add
                )
                nc.vector.tensor_scalar(
                    kkv[:], qkb[:], gbh[:, 4:5], gbh[:, 5:6], ALU.mult, ALU.add
                )

                # ---- lk token-major (batches of 4 token tiles), includes /GROUP
                lk_tok = ap.tile([128, tk * 64], bf16, tag="lk")
                for q4 in range(tk // 4):
                    ps = psA.tile([128, 4, 64], f32, tag="bank")
                    for j in range(4):
                        t = q4 * 4 + j
                        for kc in range(2):
                            nc.tensor.matmul(
                                ps[:, j, :],
                                zT[:, kc, t * 128 : (t + 1) * 128],
                                wqkp[:, kc, :],
                                start=(kc == 0),
                                stop=False,
                                skip_group_check=True,
                            )
                        nc.tensor.matmul(
                            ps[:, j, :],
                            onesr[:, 0:128],
                            bqkr,
                            start=False,
                            stop=True,
                            skip_group_check=True,
                        )
                    sq = sp.tile([128, 256], bf16, tag="sq")
                    nc.scalar.activation(
                        sq[:].rearrange("p (a b) -> p a b", a=4), ps[:], AF.Silu
                    )
                    lks = lk_tok[:, q4 * 256 : (q4 + 1) * 256]
                    nc.gpsimd.tensor_tensor(lks, sq[:], g3b[:, 0:256], ALU.mult)
                    nc.gpsimd.tensor_tensor(lks, lks, g3b[:, 256:512], ALU.add)

                # ---- v token-major (silu), batched 2 tiles per ACT
                v = ap.tile([128, tk, 512], bf16, tag="big")
                for t2 in range(tk // 2):
                    ps = psB.tile([128, 2, 512], f32, tag="pair")
                    for i in range(2):
                        t = t2 * 2 + i
                        for kc in range(2):
                            nc.tensor.matmul(
                                ps[:, i, :],
                                zT[:, kc, t * 128 : (t + 1) * 128],
                                whp[:, kc, 0:512],
                                start=(kc == 0),
                                stop=False,
                                skip_group_check=True,
                            )
                        nc.tensor.matmul(
                            ps[:, i, :],
                            onesr[:, 0:128],
                            bhv,
                            start=False,
                            stop=True,
                            skip_group_check=True,
                        )
                    nc.scalar.activation(v[:, t2 * 2 : t2 * 2 + 2, :], ps[:], AF.Silu)

                # ---- kv chain: lkv accumulates in PSUM; exclusive prefixes to SBUF
                lkv = psL.tile([64, 512], f32, tag="lkv")
                snaps = ap.tile([64, gk, 512], bf16, tag="snaps")
                nc.vector.memset(snaps[:, 0, :], 0.0)
                for g in range(gk):
                    if g > 0:
                        nc.vector.tensor_copy(snaps[:, g, :], lkv[:])
                    for i2 in range(2):
                        t = 2 * g + i2
                        nc.tensor.matmul(
                            lkv[:],
                            lk_tok[:, t * 64 : (t + 1) * 64],
                            v[:, t, :],
                            start=(g == 0 and i2 == 0),
                            stop=(g == gk - 1 and i2 == 1),
                            skip_group_check=True,
                        )
                tloc = st.tile([64, 512], f32, tag="tloc")
                nc.vector.tensor_copy(tloc[:], lkv[:])
                nc.vector.tensor_scalar_mul(tloc[:], tloc[:], pin[:])
                ccin = dr.tile([64, 512], f32, tag="ccin")
                ccout = dr.tile([64, 512], f32, tag="ccout")
                nc.sync.dma_start(ccin[:], tloc[:])
                nc.gpsimd.collective_compute(
                    "AllReduce",
                    ALU.add,
                    replica_groups=RG,
                    ins=[ccin.opt()],
                    outs=[ccout.opt()],
                )
                srecv = st.tile([64, 512], f32, tag="srecv")
                nc.sync.dma_start(srecv[:], ccout[:])
                smask = st.tile([64, 512], bf16, tag="smask")
                nc.vector.tensor_scalar_mul(smask[:], srecv[:], par[:])

                # ---- per-group quadratic attention + gate + Wo + residual
                for g in range(gk):
                    q0 = g * 256
                    simp = psA.tile([128, 384], f32, tag="bank")
                    nc.tensor.matmul(
                        simp[:, 0:256],
                        kkv[:, q0 : q0 + 128],
                        qqv[:, q0 : q0 + 256],
                        start=True,
                        stop=False,
                        skip_group_check=True,
                    )
                    nc.tensor.matmul(
                        simp[:, 0:256],
                        ident[:],
                        b2t[:, 0, :],
                        start=False,
                        stop=True,
                        skip_group_check=True,
                    )
                    nc.tensor.matmul(
                        simp[:, 256:384],
                        kkv[:, q0 + 128 : q0 + 256],
                        qqv[:, q0 + 128 : q0 + 256],
                        start=True,
                        stop=False,
                        skip_group_check=True,
                    )
                    nc.tensor.matmul(
                        simp[:, 256:384],
                        ident[:],
                        b2t[:, 1, 128:256],
                        start=False,
                        stop=True,
                        skip_group_check=True,
                    )
                    es = sp.tile([128, 384], bf16, tag="es")
                    nc.scalar.activation(es[:], simp[:], AF.Erf, scale=S0)
                    nc.vector.tensor_scalar(es[:], es[:], 0.5, 0.5, ALU.mult, ALU.add)
                    at = sp.tile([128, 384], bf16, tag="at")
                    nc.gpsimd.tensor_tensor(at[:], es[:], m01[:], ALU.mult)

                    gps = psB.tile([128, 4, 256], f32, tag="pair")
                    for j in range(4):
                        for kc in range(2):
                            nc.tensor.matmul(
                                gps[:, j, :],
                                whp[:, kc, 512 + j * 128 : 512 + (j + 1) * 128],
                                zT[:, kc, q0 : q0 + 256],
                                start=(kc == 0),
                                stop=False,
                                skip_group_check=True,
                            )
                        nc.tensor.matmul(
                            gps[:, j, :],
                            bhgr[:, j * 128 : (j + 1) * 128],
                            onesr[:, 0:256],
                            start=False,
                            stop=True,
                            skip_group_check=True,
                        )
                    gat = sp.tile([128, 4, 256], bf16, tag="gat")
                    nc.scalar.activation(gat[:], gps[:], AF.Silu)

                    qps = psB.tile([128, 4, 256], f32, tag="pair")
                    for j in range(4):
                        nc.tensor.matmul(
                            qps[:, j, :],
                            v[:, 2 * g, j * 128 : (j + 1) * 128],
                            at[:, 0:256],
                            start=True,
                            stop=False,
                            skip_group_check=True,
                        )
                        nc.tensor.matmul(
                            qps[:, j, 128:256],
                            v[:, 2 * g + 1, j * 128 : (j + 1) * 128],
                            at[:, 256:384],
                            start=False,
                            stop=False,
                            skip_group_check=True,
                        )
                        nc.tensor.matmul(
                            qps[:, j, :],
                            snaps[:, g, j * 128 : (j + 1) * 128],
                            lqv[:, q0 : q0 + 256],
                            start=False,
                            stop=False,
                            skip_group_check=True,
                        )
                        nc.tensor.matmul(
                            qps[:, j, :],
                            smask[:, j * 128 : (j + 1) * 128],
                            lqv[:, q0 : q0 + 256],
                            start=False,
                            stop=True,
                            skip_group_check=True,
                        )
                    nc.vector.tensor_tensor(gat[:], qps[:], gat[:], ALU.mult)
                    for i2 in range(2):
                        t = 2 * g + i2
                        ops_ = psA.tile([128, 512], f32, tag="bank")
                        for j in range(4):
                            nc.tensor.matmul(
                                ops_[:, 0:257],
                                gat[:, j, i2 * 128 : (i2 + 1) * 128],
                                wop[:, j, :],
                                start=(j == 0),
                                stop=False,
                                skip_group_check=True,
                            )
                        nc.tensor.matmul(
                            ops_[:, 0:257],
                            onesr[:, 0:128],
                            bor,
                            start=False,
                            stop=True,
                            skip_group_check=True,
                        )
                        nc.vector.tensor_tensor(
                            h[:, t, :], h[:, t, :], ops_[:, 0:257], ALU.add
                        )

            # ---- final: per-core token-sum of h -> [1, 257]
            fps = psA.tile([128, 512], f32, tag="bank")
            for t in range(tk):
                nc.tensor.matmul(
                    fps[0:1, 0:257],
                    onesc[:],
                    h[:, t, :],
                    start=(t == 0),
                    stop=(t == tk - 1),
                    skip_group_check=True,
                )
            hs = sp.tile([1, 257], f32, tag="hs")
            nc.vector.tensor_copy(hs[:], fps[0:1, 0:257])
            nc.sync.dma_start(out_d[:], hs[:])

    nc.compile()
    _BUILD_CACHE[key] = nc
    return nc


def prepare_inputs(inputs, nl, tk):
    """Host-side preprocessing: fold LN affine into weights, precompute rel-pos
    bias matrices / masks, pack everything in SBUF-friendly layouts."""
    f = lambda k: np.asarray(inputs[k], np.float32)
    x, W_emb, b_emb, pos_table = f("x"), f("W_emb"), f("b_emb"), f("pos_table")
    ln_g, ln_b, Wh, bh = f("ln_g"), f("ln_b"), f("Wh"), f("bh")
    Wqk, bqk, gamma, beta = f("Wqk"), f("bqk"), f("gamma"), f("beta")
    relb, Wo, bo = f("relb"), f("Wo"), f("bo")
    W_dec, b_dec = f("W_dec"), f("b_dec")

    nt = tk * 128
    rp = _rp_bucket_np(GROUP)

    whp = np.empty((nl, 128, 2, 1024), BF16NP)
    wqkp = np.empty((nl, 128, 2, 64), BF16NP)
    wop = np.empty((nl, 128, 4, 257), BF16NP)
    b2t = np.empty((nl, 128, 2, 256), np.float32)
    rows = np.empty((nl, 1, 1345), np.float32)
    gbh = np.zeros((nl, 64, 8), np.float32)
    g3b = np.empty((nl, 128, 512), BF16NP)

    for l in range(nl):
        Whp_l = ln_g[l][:, None] * Wh[l]  # [256, 1024]
        whp[l] = Whp_l.reshape(2, 128, 1024).transpose(1, 0, 2).astype(BF16NP)
        bh_eff = bh[l] + ln_b[l] @ Wh[l]  # [1024]
        Wqkp_l = ln_g[l][:, None] * Wqk[l]  # [256, 64]
        wqkp[l] = Wqkp_l.reshape(2, 128, 64).transpose(1, 0, 2).astype(BF16NP)
        bqk_eff = bqk[l] + ln_b[l] @ Wqk[l]  # [64]
        bias_l = relb[l, :, 0][rp] * math.sqrt(QKD)  # [q, k]
        b2t_l = GROUP * (bias_l.T - MU)  # [k, q]
        b2t[l] = b2t_l.reshape(2, 128, 256).transpose(1, 0, 2)
        Wo_ext = np.concatenate([Wo[l], Wo[l].sum(1, keepdims=True)], 1)  # [512,257]
        wop[l] = Wo_ext.reshape(4, 128, 257).transpose(1, 0, 2).astype(BF16NP)
        bo_ext = np.concatenate([bo[l], [bo[l].sum()]])  # [257]
        rows[l, 0, 0:512] = bh_eff[:512]
        rows[l, 0, 512:1024] = bh_eff[512:]
        rows[l, 0, 1024:1088] = bqk_eff
        rows[l, 0, 1088:1345] = bo_ext
        # heads: 0 qq, 1 lq, 2 kk, 3 lk
        gbh[l, :, 0] = gamma[l, 0]
        gbh[l, :, 1] = beta[l, 0]
        gbh[l, :, 2] = gamma[l, 1]
        gbh[l, :, 3] = beta[l, 1]
        gbh[l, :, 4] = gamma[l, 2]
        gbh[l, :, 5] = beta[l, 2]
        gbh[l, :, 6] = bqk_eff
        g3b[l, :, 0:256] = np.tile(gamma[l, 3] / GROUP, (128, 4)).astype(BF16NP)
        g3b[l, :, 256:512] = np.tile(beta[l, 3] / GROUP, (128, 4)).astype(BF16NP)

    wemb = np.concatenate([W_emb, W_emb.sum(1, keepdims=True)], 1)  # [5, 257]
    ident = np.eye(128, dtype=np.float32)
    onesr = np.ones((1, 512), np.float32)
    onesc = np.ones((128, 1), np.float32)
    m01 = np.zeros((128, 384), np.float32)
    u = np.arange(128)
    qq_ = np.arange(256)
    m01[:, 0:256] = (qq_[None, :] >= u[:, None]).astype(np.float32)
    jj = np.arange(128)
    m01[:, 256:384] = (jj[None, :] >= u[:, None]).astype(np.float32)
    m01 = m01.astype(BF16NP)

    shared = dict(
        wemb=wemb, ident=ident, onesr=onesr, onesc=onesc, m01=m01,
        whp=whp, wqkp=wqkp, wop=wop, b2t=b2t, rows=rows, gbh=gbh, g3b=g3b,
    )
    in_maps = []
    for c in range(NCORES):
        b_, hf = c // 2, c % 2
        xs = x[b_, hf * nt : (hf + 1) * nt, :]  # [nt, 5]
        pe = pos_table[hf * nt : (hf + 1) * nt, :] + b_emb  # [nt, 256]
        pos = np.concatenate([pe, pe.sum(1, keepdims=True)], 1).astype(np.float32)
        m = dict(shared)
        m["xT"] = np.ascontiguousarray(xs.T).astype(np.float32)
        m["pos"] = pos
        m["par"] = np.full((64, 1), float(hf), np.float32)
        m["pin"] = np.full((64, 1), float(1 - hf), np.float32)
        in_maps.append(m)
    return in_maps, W_dec, b_dec


def run(inputs, nl, tk, trace=False):
    from concourse.bass_utils import run_bass_kernel_spmd

    in_maps, W_dec, b_dec = prepare_inputs(inputs, nl, tk)
    nc = build(nl, tk)
    res = run_bass_kernel_spmd(
        nc, in_maps, core_ids=list(range(NCORES)), trace=trace
    )
    S = np.stack([res.results[c]["out"][0, :256] for c in range(NCORES)])
    ntot = 2 * tk * 128
    mean = (S[0::2] + S[1::2]) / float(ntot)  # [4, 256]
    y = (mean @ W_dec + b_dec).astype(np.float32)
    return y, res


def kernel(**inputs):
    y, _ = run(inputs, NL_FULL, TK_FULL)
    return y
